# revision 15
# baseline (speedup 1.0000x reference)
"""Causal MHSA prefill kernel for 8 TRN2 NeuronCores.

Sharding: data-parallel over batch (B=2) x tensor-parallel over head groups
(16 heads -> 4 groups of 4). Core c handles batch c//4, heads 4*(c%4)..+3.
Each core computes y_partial[b] = attn_out(heads) @ W_proj[:, cols]^T; the
host sums the 4 partials per batch (the "all-reduce" of the TP hint).

All matmul operands are bf16 (PSUM accumulation stays f32): same PE rate as
float32r at >=256-wide outputs, no narrow-output penalty, half the SBUF/DMA
footprint, and lower PE power.  x^T is DMA'd once and stays resident.

Per-core pipeline:
  P1  qkv = x @ W_qkv^T for this core's heads, in 2 head-pairs:
      q^T,k^T feature-major [Dh, T], v token-major [T, Dh].  The first tq
      chunk of head-pair 0 runs n-outer with 4 concurrent PSUM chains so
      matmuls start as soon as each (w[n], x[n]) DMA lands; W for the next
      head pair streams in under the previous pair's attention (bufs=2).
  P2  causal attention, the two heads of a pair interleaved in one j-loop.
      Both heads' S^T blocks land in one 2-bank PSUM tile [128,2,512] so a
      SINGLE ACT exp instruction covers both (ACT was the phase bottleneck
      at one exp per block: 686ns exec + 158ns issue overhead each).  Row
      sums via ones-vector matmuls into one shared PSUM bank (rows 0/32),
      out^T[dh,tq] = v.T @ P^T in PSUM.  The rowsum/AV flush for block j is
      deferred by TWO j-steps (carried across chain boundaries) so the PE
      never heads the FIFO waiting on an exp; normalization (PE broadcast
      of row sums, reciprocal, multiply on the attnT chunk) trails by a
      chain.
  P3  y_partial = attn^T.T @ W_proj_cols^T, W_proj prefetched during P2;
      chain pairs share a 2-bank PSUM tile, one DVE copy + one DMA per
      [128,1024] pair.
"""

import sys

if "/opt/trn_rl_repo" not in sys.path:
    sys.path.insert(0, "/opt/trn_rl_repo")

import ml_dtypes
import numpy as np

import concourse.bacc as bacc
import concourse.tile as tile
from concourse import mybir
from concourse.bass import ts
from concourse.bass_utils import run_bass_kernel_spmd

B, T, D = 2, 2048, 2048
H, DH = 16, 128
HEADS_PER_CORE = 4
N_CORES = 8
NT = T // 128           # 16 token tiles
ND = D // 128           # 16 contraction tiles
NC_CHUNK = T // 512     # 4 tq/t chunks of 512
SCALE = 1.0 / np.sqrt(np.float32(DH))
NEG = -1.0e30

F32 = mybir.dt.float32
F32R = mybir.dt.float32r
BF16 = mybir.dt.bfloat16
EXP = mybir.ActivationFunctionType.Exp
COPY = mybir.ActivationFunctionType.Copy

_compiled = None


def _build():
    nc = bacc.Bacc("TRN2", target_bir_lowering=False, debug=False,
                   num_devices=N_CORES)

    xT = nc.dram_tensor("xT", [D, T], BF16, kind="ExternalInput")
    # per head-pair blocks of W_qkv^T: cols = [q(2x128) | k(2x128) | v(2x128)]
    wT = nc.dram_tensor("wT", [2, D, 768], BF16, kind="ExternalInput")
    wpT = nc.dram_tensor("wpT", [HEADS_PER_CORE * DH, D], BF16,
                         kind="ExternalInput")
    mask = nc.dram_tensor("mask", [128, 128], F32, kind="ExternalInput")
    ones = nc.dram_tensor("ones", [1, 128], F32, kind="ExternalInput")
    ones16 = nc.dram_tensor("ones16", [128, 1], BF16, kind="ExternalInput")
    y = nc.dram_tensor("y", [T, D], BF16, kind="ExternalOutput")

    xT_r = xT.ap().rearrange("(n p) t -> p n t", p=128)

    def s2tile(name):
        return None  # placeholder (rebound below)

    with tile.TileContext(nc) as tc:
        with (
            tc.tile_pool(name="persist", bufs=1) as persist,
            tc.tile_pool(name="wpool", bufs=2) as wpool,
            tc.tile_pool(name="work", bufs=2) as work,
            tc.tile_pool(name="ybuf", bufs=2) as ybuf,
            tc.tile_pool(name="ps2", bufs=2, space="PSUM") as ps2,
            tc.tile_pool(name="ps1", bufs=1, space="PSUM") as ps1,
        ):
            def s2tile(name):
                # [128, 2, 512] f32 = two adjacent PSUM banks
                return ps2.tile([128, 2, 512], F32, tag="s2", bufs=2,
                                name=name)

            mask_sb = persist.tile([128, 128], F32, tag="mask")
            nc.sync.dma_start(out=mask_sb, in_=mask.ap())
            ones_col = persist.tile([128, 1], BF16, tag="ones_col")
            nc.gpsimd.dma_start(out=ones_col, in_=ones16.ap())
            ones_row = persist.tile([1, 128], F32R, tag="ones_row")
            nc.gpsimd.dma_start(out=ones_row, in_=ones.ap())

            x_sb = persist.tile([128, ND, T], BF16, tag="x")
            qk_tags = ["qT0", "qT1", "kT0", "kT1"]
            attnT = [persist.tile([128, T], BF16, tag=f"attnT{i}",
                                  name=f"attnT{i}")
                     for i in range(HEADS_PER_CORE)]
            wp = [persist.tile([128, D], BF16, tag=f"wp{e}", name=f"wp{e}")
                  for e in range(4)]
            wpT_ap = wpT.ap()

            for hp in range(2):
                w_sb = wpool.tile([128, ND, 768], BF16, tag="w")
                wT_r = wT.ap()[hp].rearrange("(n p) e -> p n e", p=128)
                if hp == 0:
                    # interleave w[n] / x[n, chunk0] so the first n-outer
                    # qk chains start as soon as each pair lands; the other
                    # x chunks stream under compute
                    for n in range(ND):
                        nc.gpsimd.dma_start(out=w_sb[:, n, :],
                                            in_=wT_r[:, n, :])
                        nc.gpsimd.dma_start(out=x_sb[:, n, 0:512],
                                            in_=xT_r[:, n, 0:512])
                    for tcc in range(1, NC_CHUNK):
                        for n in range(ND):
                            nc.gpsimd.dma_start(
                                out=x_sb[:, n, ts(tcc, 512)],
                                in_=xT_r[:, n, ts(tcc, 512)])
                else:
                    # second head pair's weights stream during P1/P2 of the
                    # first; W_proj right behind them (used only in P3)
                    for n in range(ND):
                        nc.gpsimd.dma_start(out=w_sb[:, n, :],
                                            in_=wT_r[:, n, :])
                    for e in range(4):
                        nc.gpsimd.dma_start(out=wp[e],
                                            in_=wpT_ap[ts(e, 128), :])

                qk = [persist.tile([128, T], BF16, tag=t, name=f"{t}_{hp}")
                      for t in qk_tags]
                v_sb = persist.tile([128, NT, 256], BF16, tag="v")

                # ---- P1: qkv for this head pair ----
                for tci in range(NC_CHUNK):
                    if hp == 0 and tci == 0:
                        # n-outer with 4 concurrent accumulators: each
                        # (w[n], x[n]) is consumed right as its DMA lands
                        pse = [s2tile("pse0"), s2tile("pse1")]
                        for n in range(ND):
                            for et in range(4):
                                nc.tensor.matmul(
                                    pse[et // 2][:, et % 2, :],
                                    w_sb[:, n, ts(et, 128)],
                                    x_sb[:, n, 0:512],
                                    start=(n == 0), stop=(n == ND - 1))
                        for et in range(4):
                            nc.scalar.activation(qk[et][:, 0:512],
                                                 pse[et // 2][:, et % 2, :],
                                                 COPY)
                    else:
                        for eh in range(2):  # (q0,q1) then (k0,k1)
                            ps = s2tile("ps_qk")
                            for et2 in range(2):
                                et = eh * 2 + et2
                                for n in range(ND):
                                    nc.tensor.matmul(
                                        ps[:, et2, :],
                                        w_sb[:, n, ts(et, 128)],
                                        x_sb[:, n, ts(tci, 512)],
                                        start=(n == 0), stop=(n == ND - 1))
                            for et2 in range(2):
                                nc.scalar.activation(
                                    qk[eh * 2 + et2][:, ts(tci, 512)],
                                    ps[:, et2, :], COPY)
                    for th in range(2):  # v, token-major, pairs of 128-tok
                        ps = s2tile("ps_v")
                        for tt2 in range(2):
                            tt = th * 2 + tt2
                            for n in range(ND):
                                nc.tensor.matmul(
                                    ps[:, tt2, 0:256],
                                    x_sb[:, n, tci * 512 + tt * 128:
                                         tci * 512 + tt * 128 + 128],
                                    w_sb[:, n, 512:768],
                                    start=(n == 0), stop=(n == ND - 1))
                        for tt2 in range(2):
                            nc.scalar.activation(
                                v_sb[:, tci * 4 + th * 2 + tt2, :],
                                ps[:, tt2, 0:256], COPY)

                # ---- P2: attention, both heads interleaved ----
                rs_t = ps1.tile([64, 512], F32, tag="rs")  # rows 0 / 32

                def emit_tail(tail):
                    rs_row, toT, ttci = tail
                    ps_b = ps1.tile([128, 512], F32, tag="b")
                    nc.tensor.matmul(ps_b, ones_row, rs_row,
                                     start=True, stop=True)
                    with nc.allow_low_precision(
                            reason="bf16 softmax normalization"):
                        nc.vector.reciprocal(ps_b, ps_b)
                        nc.vector.tensor_mul(toT[:, ts(ttci, 512)],
                                             toT[:, ts(ttci, 512)], ps_b)

                # pend[i]: deferred flushes for head i (depth 2):
                # (p2_sb, off, w, j, first, last, ps_o, oT, tci)
                pend = [[], []]
                tails = []

                def flush(i):
                    p2_sb, off, w, j, first, last, ps_o, oT, tci = \
                        pend[i].pop(0)
                    nc.tensor.matmul(rs_t[32 * i:32 * i + 1, off:off + w],
                                     ones_col, p2_sb[:, i, 0:w],
                                     start=first, stop=last,
                                     skip_group_check=True)
                    nc.tensor.matmul(ps_o[:, off:off + w],
                                     v_sb[:, j, ts(i, 128)],
                                     p2_sb[:, i, 0:w],
                                     start=first, stop=last)
                    if last:
                        rs_sb = work.tile([1, 512], F32R, tag=f"rs{i}",
                                          bufs=1)
                        nc.vector.tensor_copy(rs_sb,
                                              rs_t[32 * i:32 * i + 1, :])
                        nc.vector.tensor_copy(oT[:, ts(tci, 512)], ps_o)
                        tails.append((rs_sb, oT, tci))

                for tci in range(NC_CHUNK):
                    jmax = tci * 4 + 4
                    ps_o = [ps2.tile([128, 512], F32, tag="o",
                                     name=f"ps_o{oi}")
                            for oi in range(2)]
                    for j in range(jmax):
                        off = 0 if j < tci * 4 else (j - tci * 4) * 128
                        w = 512 - off
                        ps_s2 = s2tile("ps_s2")
                        for i in range(2):
                            nc.tensor.matmul(
                                ps_s2[:, i, 0:w], qk[2 + i][:, ts(j, 128)],
                                qk[i][:, tci * 512 + off:(tci + 1) * 512],
                                start=True, stop=True)
                            if len(pend[i]) >= 2:
                                flush(i)
                        if tails and j in (2, 3):
                            emit_tail(tails.pop(0))
                        if j >= tci * 4:
                            for i in range(2):
                                nc.vector.tensor_add(ps_s2[:, i, 0:128],
                                                     ps_s2[:, i, 0:128],
                                                     mask_sb)
                        p2_sb = work.tile([128, 2, 512], BF16, tag="P",
                                          bufs=3)
                        with nc.allow_low_precision(
                                reason="bf16 attention probabilities"):
                            nc.scalar.activation(p2_sb[:, :, 0:w],
                                                 ps_s2[:, :, 0:w], EXP,
                                                 scale=float(SCALE))
                        for i in range(2):
                            pend[i].append((p2_sb, off, w, j, j == 0,
                                            j == jmax - 1,
                                            ps_o[i], attnT[hp * 2 + i],
                                            tci))
                # drain carried flushes, then remaining tails
                while pend[0] or pend[1]:
                    for i in range(2):
                        if pend[i]:
                            flush(i)
                while tails:
                    emit_tail(tails.pop(0))

            # ---- P3: y_partial = attn^T.T @ wpT ----
            for m in range(NT):
                for nh in range(NC_CHUNK // 2):
                    y_sb = ybuf.tile([128, 2, 512], BF16, tag="y", bufs=3)
                    ps = s2tile("ps_y")
                    for nck2 in range(2):
                        for e in range(4):
                            nc.tensor.matmul(
                                ps[:, nck2, :], attnT[e][:, ts(m, 128)],
                                wp[e][:, ts(nh * 2 + nck2, 512)],
                                start=(e == 0), stop=(e == 3))
                    with nc.allow_low_precision(
                            reason="bf16 partial-sum output"):
                        nc.vector.tensor_copy(y_sb, ps)
                    nc.sync.dma_start(
                        out=y.ap()[ts(m, 128), ts(nh, 1024)].rearrange(
                            "p (a b) -> p a b", a=2),
                        in_=y_sb)

    nc.compile()
    return nc


def _get_compiled():
    global _compiled
    if _compiled is None:
        _compiled = _build()
    return _compiled


def _shard_inputs(x, W_qkv, W_proj):
    """Build the 8 per-core input maps (host-side transposes/slices)."""
    bf16 = np.dtype(ml_dtypes.bfloat16)
    x = np.asarray(x, dtype=np.float32)
    W_qkv = np.asarray(W_qkv, dtype=np.float32)
    W_proj = np.asarray(W_proj, dtype=np.float32)

    mask = np.where(np.arange(128)[None, :] >= np.arange(128)[:, None],
                    np.float32(0.0), np.float32(NEG))  # [tk, tq]

    in_maps = []
    for c in range(N_CORES):
        b, g = divmod(c, HEADS_PER_CORE)
        xT = np.ascontiguousarray(x[b].T).astype(bf16)
        wt = np.empty((2, D, 768), dtype=bf16)
        for hp in range(2):
            rows = []
            for blk in range(3):  # q, k, v row blocks of W_qkv
                h0 = (4 * g + 2 * hp) * DH
                rows.append(W_qkv[blk * D + h0: blk * D + h0 + 2 * DH])
            wt[hp] = np.concatenate(rows, axis=0).T.astype(bf16)
        cols = slice(4 * g * DH, 4 * g * DH + HEADS_PER_CORE * DH)
        wpT = np.ascontiguousarray(W_proj[:, cols].T).astype(bf16)
        in_maps.append({"xT": xT, "wT": wt, "wpT": wpT, "mask": mask,
                        "ones": np.ones((1, 128), dtype=np.float32),
                        "ones16": np.ones((128, 1), dtype=bf16)})
    return in_maps


def kernel(x, W_qkv, W_proj, step, trace=False, trace_cores=None):
    nc = _get_compiled()
    in_maps = _shard_inputs(x, W_qkv, W_proj)
    res = run_bass_kernel_spmd(nc, in_maps, list(range(N_CORES)),
                               trace=trace, trace_cores=trace_cores)
    y = np.zeros((B, T, D), dtype=np.float32)
    for c in range(N_CORES):
        y[c // HEADS_PER_CORE] += np.asarray(res.results[c]["y"],
                                             dtype=np.float32)
    kernel.last_exec_time_ns = res.exec_time_ns
    kernel.last_result = res
    return y
